# revision 59
# baseline (speedup 1.0000x reference)
"""Causal multi-head self-attention on 8 trn2 NeuronCores (bf16, pipelined).

Sharding: core c = (batch, head_group): batch = c // 4, heads = [4*(c%4) .. 4*(c%4)+3].
Each core computes the QKV projection for its batch + 4 heads, causal attention,
and a row-parallel slice of the output projection; the host sums the 4 partial
outputs per batch element.

Device design notes:
 - all matmul operands are bf16: the PE streams bf16 moving operands at
   1 col/cycle @2.4GHz vs 2 bytes/cycle for fp32r. PSUM accumulation stays fp32.
 - every DRAM tensor is laid out so each dma_start moves one fully
   contiguous block per partition (xt pre-tiled [ci][p][kt][it], weights
   [p][kt][f], outputs [ci][op][p][f]); host does the rearranges.
 - a run of dummy 128-col matmuls on a memset tile warms the PE HAM clock
   gate (1.2 -> 2.4 GHz) during the initial weight/x DMA wait.
 - x is passed transposed so both projection matmuls have the contraction
   dim (channels) on partitions.
 - attention scores are computed transposed: ST[j, i] = (k_j . q_i)/8 with j on
   partitions. The two heads of a pair run as one packed PE slot via
   tile_position (0,0)/(64,0) row tiling (K=64 each).
 - softmax denominator comes from a ones-column appended to V (M=65 PV matmul);
   it is inverted with DVE reciprocal and broadcast across 64 partitions with a
   ones-column PE matmul.
 - no max-subtraction in softmax: scores are ~N(0,1), exp is safe in fp32 PSUM.
 - causal blocks are exact at 128-column granularity; diagonal triangles are
   zeroed by gpsimd affine_select after exp.
 - emission is planned with a coarse per-engine time model: projection and
   output-projection matmul chunks are held in a filler queue and emitted
   wherever the PE queue would otherwise block on a semaphore (exp results,
   PSUM tile reuse), so the PE pipeline stays dense.
"""

import ml_dtypes
import numpy as np
from collections import deque
from contextlib import ExitStack

import concourse.bass as bass
from concourse import bacc
import concourse.mybir as mybir
import concourse.tile as tile
from concourse.bass_utils import run_bass_kernel_spmd

bf16 = ml_dtypes.bfloat16

B, T, D, H, HD = 2, 2048, 1024, 16, 64
NCORES = 8
HPC = 4  # heads per core

f32 = mybir.dt.float32
R = mybir.dt.bfloat16
Exp = mybir.ActivationFunctionType.Exp
MUL = mybir.AluOpType.mult

LAST_RESULTS = None  # BassKernelResults of the most recent kernel() call

N_WARM = 15  # N=512 HAM warmup matmuls: ~6-9 cold (427ns) + rest warm (216ns)


def build_bass(t=T):
    """Build the per-core Bass program (SPMD: same program, different data)."""
    assert t % 512 == 0
    nci = t // 512      # 512-wide i-chunks
    njt_tot = t // 128  # 128-wide j-tiles

    nc = bacc.Bacc("TRN2", target_bir_lowering=False)
    xt = nc.dram_tensor("xt", [nci, 128, 8, 512], R, kind="ExternalInput")
    # wqk split by pair-half (ft 0/1 vs 2/3): pass A only needs half 0, so
    # all of pass A's input rides first-FIFO-slot DMAs on the three queues.
    wqk = nc.dram_tensor("wqk", [2, 128, 8, 256], R, kind="ExternalInput")
    wv = nc.dram_tensor("wv", [128, 8, 256], R, kind="ExternalInput")
    wo = nc.dram_tensor("wo", [128, 2, D], R, kind="ExternalInput")
    outp = nc.dram_tensor("outp", [nci, 4, 128, 1024], R, kind="ExternalOutput")
    # pair-0-only output-projection partials of the last i-chunk: computed
    # during the final pair's attention windows, summed with outp on host.
    outp2 = nc.dram_tensor("outp2", [4, 128, 1024], R, kind="ExternalOutput")

    with ExitStack() as ctx:
        tc = ctx.enter_context(tile.TileContext(nc))
        persist = ctx.enter_context(tc.tile_pool(name="persist", bufs=1))
        xin_pool = ctx.enter_context(tc.tile_pool(name="xin", bufs=2))
        exps = ctx.enter_context(tc.tile_pool(name="exps", bufs=4))
        otn_pool = ctx.enter_context(tc.tile_pool(name="otn", bufs=8))
        rcp_pool = ctx.enter_context(tc.tile_pool(name="rcp", bufs=2))
        rcpb_pool = ctx.enter_context(tc.tile_pool(name="rcpb", bufs=2))
        osb_pool = ctx.enter_context(tc.tile_pool(name="osb", bufs=3))
        ppsum = ctx.enter_context(tc.tile_pool(name="ppsum", bufs=2, space="PSUM"))
        spsum = ctx.enter_context(tc.tile_pool(name="spsum", bufs=2, space="PSUM"))
        pvpsum = ctx.enter_context(tc.tile_pool(name="pvpsum", bufs=1, space="PSUM"))

        # ---- coarse per-engine completion-time estimates (ns) -----------
        est = {"pe": 0.0, "act": 0.0, "dve": 0.0}

        def e_pe(n_cols, dep=0.0, ovh=15.0):
            est["pe"] = max(est["pe"], dep) + n_cols / 2.4 + ovh
            return est["pe"]

        def e_act(fd, dep=0.0):
            est["act"] = max(est["act"], dep + 120.0) + 290.0 + fd / 1.2
            return est["act"]

        def e_dve(fd, dep=0.0, ovh=165.0):
            est["dve"] = max(est["dve"], dep + 120.0) + ovh + fd / 0.96
            return est["dve"]

        def e_dma(nbytes_total):
            return max(est["pe"], est["dve"]) + 2000.0 + nbytes_total / 350.0

        # ---- weights / constants ---------------------------------------
        wqk_sb = persist.tile([128, 2, 8, 256], R, tag="wqk_sb", name="wqk_sb")
        wv_sb = persist.tile([128, 8, 256], R, tag="wv_sb", name="wv_sb")
        wo_sb = persist.tile([128, 2, D], R, tag="wo_sb", name="wo_sb")
        ones_sb = persist.tile([128, 64], R, tag="ones_sb", name="ones_sb")
        warm_sb = persist.tile([128, 512], R, tag="warm_sb", name="warm_sb")

        v_sb = persist.tile([128, njt_tot, HPC, HD + 1], R, tag="v_sb", name="v_sb")

        # qk_sb[ft][ci]: ft 0=q pair0, 1=k pair0, 2=q pair1, 3=k pair1
        # each tile [128, 512]: partitions 0:64 head A dims, 64:128 head B dims
        qk_sb = [[persist.tile([128, 512], R, tag=f"qk_{ft}_{ci}", name=f"qk_{ft}_{ci}")
                  for ci in range(nci)] for ft in range(4)]

        xin_tiles = {}
        xin_done = {}

        def issue_xin(ci, eng):
            if ci in xin_tiles or ci >= nci:
                return
            xin = xin_pool.tile([128, 8, 512], R, tag="xin", name="xin")
            xin_tiles[ci] = xin
            eng.dma_start(out=xin, in_=xt[ci])
            xin_done[ci] = e_dma(128 * 8 * 512 * 2)

        # HAM warmup: memset a tile, then dummy matmuls keep the PE busy
        # while the first weight/x DMAs are in flight.  warm_sb is memset on
        # gpsimd, whose preamble finishes earliest, so the first warm matmul
        # is not gated on the vector queue's slower startup.
        nc.gpsimd.memset(warm_sb, 0.0)
        nc.vector.memset(ones_sb, 1.0)

        # first-needed inputs first, interleaved across the three DMA-capable
        # queues in kt order (proj(0) consumes wqk+xin0 kt-laddered, kt-major).
        # Everything not needed in the first ~15us (wv, wo, xin1) is issued
        # later from a busy queue so it cannot steal SDMA bandwidth from the
        # critical proj(0) feed.
        # half-tensor slabs: 4KB per partition keeps the SDMA descriptors
        # efficient (2KB-per-partition slabs measured only ~160GB/s
        # aggregate); the kt0-3 halves land first and feed pass A's ladder.
        # Per-ring DMAs serialize through their ~2us completion receipt, so
        # the three transfers pass A needs (wqk half 0, both xin0 halves)
        # each ride the first FIFO slot of a queue; wqk half 1 and wv are
        # needed several us later and queue second.
        # all critical loads stay on the two HWDGE queues (sync/scalar):
        # gpsimd(SWDGE)-issued DMAs fire their completion ~4-5us after the
        # transfer instead of ~2us.
        xin0 = xin_pool.tile([128, 8, 512], R, tag="xin", name="xin")
        xin_tiles[0] = xin0
        nc.sync.dma_start(out=wqk_sb[:, 0], in_=wqk[0])
        nc.scalar.dma_start(out=xin0[:, 0:4, :], in_=xt[0, :, 0:4, :])
        nc.sync.dma_start(out=xin0[:, 4:8, :], in_=xt[0, :, 4:8, :])
        nc.sync.dma_start(out=wqk_sb[:, 1], in_=wqk[1])
        xin_done[0] = e_dma(1024 * 1024 + 512 * 1024)
        # wv is first needed by the ci=0 v-chunks around ~18us, xin1 by the
        # proj(1) filler during the pair-(0,1) attention windows (~28us).
        nc.scalar.dma_start(out=wv_sb, in_=wv[:])
        wv_done = e_dma(128 * 8 * 256 * 2)
        issue_xin(1, nc.scalar)
        # third-in-FIFO behind xin-h0 and wv with serialized ~2us receipts.
        xin_done[1] = 20000.0

        wps = ppsum.tile([128, 512], f32, tag="mm512", name="pp")
        for _ in range(N_WARM):
            nc.tensor.matmul(wps, lhsT=warm_sb[:, 0:128], rhs=warm_sb,
                             start=True, stop=True)

        def pad_ldw(target, cap=5):
            # standalone background-buffer weight loads: free PE-side
            # activity (no PSUM write, no hazards) to keep the HAM clock
            # gate warm through short exp-paced windows late in the kernel.
            n = 0
            while est["pe"] < target - 300.0 and n < cap:
                nc.tensor.ldweights(weights=warm_sb[:, 0:128])
                est["pe"] += 110.0
                n += 1

        def emit_warm_dummies(n):
            # keep the PE HAM clock gate warm across a known stall window:
            # dummy matmuls into a scratch spsum tile (its previous readers
            # are long done, so only same-engine WAW ordering applies).
            dmy = spsum.tile([128, 2, 512], f32, tag="sp", name="sp")
            for _ in range(n):
                e_pe(512, ovh=10.0)
                nc.tensor.matmul(dmy[:, 0, :], lhsT=warm_sb[:, 0:128],
                                 rhs=warm_sb, start=True, stop=True)

        # v with appended ones column: [j_in_tile, jt, head, 65]
        nc.vector.tensor_copy(
            out=v_sb[:, :, :, HD],
            in_=ones_sb[:, 0].to_broadcast([128, njt_tot, HPC]),
        )

        qk_done = {}
        v_done = {}
        otn_tiles = {}
        otn_done = {}

        def emit_qk_pass_ci0():
            # kt-major over the first two qk chunks of ci=0: each arriving
            # (wqk, xin0) kt-slab immediately feeds 2x512 matmul cols, so the
            # PE consumption rate matches the DMA arrival rate during the
            # initial ladder crawl instead of stalling chunk-by-chunk.
            xin = xin_tiles[0]
            ps = [ppsum.tile([128, 512], f32, tag="mm512", name="pp")
                  for _ in range(2)]
            for kt in range(8):
                slab = 13000.0 + (kt // 4) * 1400.0
                for ft in range(2):
                    e_pe(512, dep=(slab if ft == 0 else 0.0))
                    nc.tensor.matmul(
                        ps[ft],
                        lhsT=wqk_sb[:, 0, kt, ft * 128:(ft + 1) * 128],
                        rhs=xin[:, kt, :],
                        start=(kt == 0), stop=(kt == 7),
                    )
            for ft in range(2):
                nc.vector.tensor_copy(out=qk_sb[ft][0], in_=ps[ft])
                qk_done[(ft, 0)] = e_dve(512, dep=est["pe"])

        def emit_qk_chunk(ci, ft):
            xin = xin_tiles[ci]
            ps = ppsum.tile([128, 512], f32, tag="mm512", name="pp")
            for kt in range(8):
                e_pe(512, dep=(xin_done.get(ci, 0.0) if kt == 0 else 0.0))
                nc.tensor.matmul(
                    ps,
                    lhsT=wqk_sb[:, ft // 2, kt, (ft % 2) * 128:(ft % 2 + 1) * 128],
                    rhs=xin[:, kt, :],
                    start=(kt == 0), stop=(kt == 7),
                )
            nc.vector.tensor_copy(out=qk_sb[ft][ci], in_=ps)
            qk_done[(ft, ci)] = e_dve(512, dep=est["pe"])

        def emit_v_chunk(ci, it):
            xin = xin_tiles[ci]
            jt = ci * 4 + it
            ps = ppsum.tile([128, 512], f32, tag="mm512", name="pp")
            for kt in range(8):
                e_pe(256, dep=(max(xin_done.get(ci, 0.0), wv_done)
                               if kt == 0 else 0.0))
                nc.tensor.matmul(
                    ps[:, 0:256],
                    lhsT=xin[:, kt, it * 128:(it + 1) * 128],
                    rhs=wv_sb[:, kt, :],
                    start=(kt == 0), stop=(kt == 7),
                )
            nc.vector.tensor_copy(
                out=v_sb[:, jt, :, 0:HD],
                in_=ps[:, 0:256].rearrange("p (h d) -> p h d", h=HPC),
            )
            v_done[jt] = e_dve(256, dep=est["pe"])

        def emit_outproj_chunk(ci, op):
            # one chunk = two adjacent ot halves -> one 256KB contiguous DMA
            osb = osb_pool.tile([128, 1024], R, tag="osb", name="osb")
            for half in range(2):
                ot = 2 * op + half
                ps = ppsum.tile([128, 512], f32, tag="mm512", name="pp")
                for pair in range(2):
                    e_pe(512, dep=(otn_done.get(ci, 0.0)
                                   if (pair == 0 and half == 0) else 0.0))
                    nc.tensor.matmul(
                        ps,
                        lhsT=wo_sb[:, pair, ot * 128:(ot + 1) * 128],
                        rhs=otn_tiles[(ci, pair)],
                        start=(pair == 0), stop=(pair == 1),
                    )
                dst = osb[:, half * 512:(half + 1) * 512]
                nc.vector.tensor_copy(out=dst, in_=ps)
                e_dve(512, dep=est["pe"])
            dma_eng = nc.sync if op % 2 == 0 else nc.gpsimd
            dma_eng.dma_start(out=outp[ci, op], in_=osb)

        otn_p0_done = {}

        def emit_outproj_pair(ci, op, pair):
            # last-chunk output projection split by pair: pair-0 partials run
            # during the final pair's attention windows, pair-1 partials are
            # all that remains after the last normalize; host sums the two.
            osb = osb_pool.tile([128, 1024], R, tag="osb", name="osb")
            for half in range(2):
                ot = 2 * op + half
                ps = ppsum.tile([128, 512], f32, tag="mm512", name="pp")
                e_pe(512, dep=((otn_p0_done.get(ci, 0.0) if pair == 0
                                else otn_done.get(ci, 0.0))
                               if half == 0 else 0.0))
                nc.tensor.matmul(
                    ps,
                    lhsT=wo_sb[:, pair, ot * 128:(ot + 1) * 128],
                    rhs=otn_tiles[(ci, pair)],
                    start=True, stop=True,
                )
                dst = osb[:, half * 512:(half + 1) * 512]
                if pair == 1 and half == 1:
                    # tail: scalar is idle once the last exp is done -
                    # alternating evacuations halves the serialized drain.
                    nc.scalar.activation(
                        out=dst, in_=ps,
                        func=mybir.ActivationFunctionType.Copy,
                    )
                    est["act"] += 600.0
                else:
                    nc.vector.tensor_copy(out=dst, in_=ps)
                    e_dve(512, dep=est["pe"])
            if pair == 0:
                dma_eng = nc.sync if op % 2 == 0 else nc.gpsimd
                dma_eng.dma_start(out=outp2[op], in_=osb)
            else:
                dma_eng = [nc.sync, nc.gpsimd, nc.scalar][op % 3]
                dma_eng.dma_start(out=outp[ci, op], in_=osb)

        # ---- filler queue of PE chunks ---------------------------------
        # each entry: [key, ready_fn, emit_fn]; emitted at most once.
        fill_q = deque()
        emitted = set()

        INF = float("inf")

        def push_proj(ci):
            for ft in range(4):
                fill_q.append((("qk", ci, ft),
                               lambda ci=ci: xin_done.get(ci, INF),
                               lambda ci=ci, ft=ft: emit_qk_chunk(ci, ft)))
            for it in range(4):
                fill_q.append((("v", ci, it),
                               lambda ci=ci: xin_done.get(ci, INF),
                               lambda ci=ci, it=it: emit_v_chunk(ci, it)))

        def push_outproj(ci):
            if ci == nci - 1:
                for op in range(4):
                    fill_q.append((("op1", ci, op),
                                   lambda ci=ci: otn_done.get(ci, 0.0),
                                   lambda ci=ci, op=op:
                                   emit_outproj_pair(ci, op, 1)))
            else:
                for op in range(4):
                    fill_q.append((("op", ci, op),
                                   lambda ci=ci: otn_done.get(ci, 0.0),
                                   lambda ci=ci, op=op:
                                   emit_outproj_chunk(ci, op)))

        def pull(key):
            """Force-emit a specific chunk now (if not already emitted)."""
            if key in emitted:
                return
            for i, (k, _, emit) in enumerate(fill_q):
                if k == key:
                    del fill_q[i]
                    emitted.add(k)
                    emit()
                    return

        def pull_filler(target, keep=0):
            """Emit ready filler chunks until est pe time reaches target."""
            while len(fill_q) > keep and est["pe"] < target:
                picked = None
                for i, (k, ready, _) in enumerate(fill_q):
                    if ready() <= est["pe"] + 100.0:
                        picked = i
                        break
                if picked is None:
                    break
                k, _, emit = fill_q[picked]
                del fill_q[picked]
                emitted.add(k)
                emit()

        # ---- attention -------------------------------------------------
        def emit_scores(ci, pair, jt):
            pull(("qk", ci, 2 * pair))
            pull(("qk", jt // 4, 2 * pair + 1))
            s = max(0, (jt - 4 * ci)) * 128
            qtile = qk_sb[2 * pair][ci]
            ktile = qk_sb[2 * pair + 1][jt // 4]
            ksl = ktile[:, (jt % 4) * 128:(jt % 4 + 1) * 128]
            sp = spsum.tile([128, 2, 512], f32, tag="sp", name="sp")
            dep = max(qk_done.get((2 * pair, ci), 0.0),
                      qk_done.get((2 * pair + 1, jt // 4), 0.0))
            nc.tensor.matmul(
                sp[:, 0, s:512],
                lhsT=ksl[0:64, :],
                rhs=qtile[0:64, s:512],
                tile_position=(0, 0),
            )
            nc.tensor.matmul(
                sp[:, 1, s:512],
                lhsT=ksl[64:128, :],
                rhs=qtile[64:128, s:512],
                tile_position=(64, 0),
            )
            sp_done = e_pe(512 - s, dep=dep, ovh=120.0)
            return sp, s, sp_done

        def emit_exp(ci, pair, jt, sp, s, sp_done):
            ex = exps.tile([128, 2, 512], R, tag="ex", name="ex")
            nc.scalar.activation(
                out=ex[:, :, s:512], in_=sp[:, :, s:512],
                func=Exp, scale=0.125,
            )
            ex_done = e_act(2 * (512 - s), dep=sp_done)
            if jt - 4 * ci >= 0:
                # zero the diagonal triangle, both heads in one strided call
                nc.gpsimd.affine_select(
                    out=ex[:, :, s:s + 128],
                    in_=ex[:, :, s:s + 128],
                    compare_op=mybir.AluOpType.is_ge,
                    fill=0.0,
                    base=0,
                    channel_multiplier=-1,
                    pattern=[[0, 2], [1, 128]],
                )
                ex_done += 480.0
            return ex, ex_done

        def emit_pv(ci, pair, jt, njt, pv, ex, s, ex_done):
            for hh in range(2):
                e_pe(512 - s, dep=(max(ex_done, v_done.get(jt, 0.0))
                                   if hh == 0 else 0.0), ovh=8.0)
                nc.tensor.matmul(
                    pv[:, hh, s:512],
                    lhsT=v_sb[:, jt, 2 * pair + hh, :],
                    rhs=ex[:, hh, s:512],
                    start=(jt == 0), stop=(jt == njt - 1),
                )

        pv_free = [0.0]

        def emit_attn_pair(ci, pair):
            njt = 4 * (ci + 1)
            pv = pvpsum.tile([HD + 1, 2, 512], f32, tag="pv", name="pv")
            sps = {0: emit_scores(ci, pair, 0)}
            exs = {}
            # hold a couple of filler chunks back during the final pair so the
            # PE is not left fully idle (HAM re-throttle to 1.2GHz) in the
            # last normalize window before the tail output projection.
            keep = 2 if (ci == nci - 1 and pair == 1) else 0
            for jt in range(njt):
                sp, s, sp_done = sps.pop(jt)
                exs[jt] = (emit_exp(ci, pair, jt, sp, s, sp_done), s)
                if ci == 0 and pair == 0 and jt == 2:
                    nc.gpsimd.dma_start(out=wo_sb, in_=wo[:])
                if jt + 1 < njt:
                    sps[jt + 1] = emit_scores(ci, pair, jt + 1)
                # PV runs one jt behind its exp so the PE queue head never
                # blocks on the activation; filler fills the remaining slack.
                for j in ([jt - 1, jt] if jt == njt - 1 else [jt - 1]):
                    if j < 0:
                        continue
                    (ex, ex_done), s_j = exs.pop(j)
                    target = ex_done
                    if j == 0:
                        target = max(target, pv_free[0] + 1500.0)
                    pull_filler(target, keep=keep)
                    pull(("v", j // 4, j % 4))
                    emit_pv(ci, pair, j, njt, pv, ex, s_j, ex_done)

            if ci == nci - 1 and pair == 1:
                # final normalize window: the reserved pair-0 outproj chunks
                # plus a few dummies keep the HAM gate warm into the tail.
                emit_warm_dummies(3)
            # normalize: den row -> bf16 -> PE ones-broadcast -> reciprocal
            # -> scale.  (DMA cannot stride-0 broadcast across partitions.)
            den = rcp_pool.tile([1, 2, 512], R, tag="den", name="den")
            nc.vector.tensor_copy(out=den, in_=pv[HD:HD + 1, :, :])
            den_done = e_dve(1024, dep=est["pe"])
            otn = otn_pool.tile([128, 512], R, tag="otn", name="otn")
            otn_tiles[(ci, pair)] = otn
            # keep the PE fed while the den cast completes: the first
            # broadcast matmul below would otherwise block the queue head.
            pull_filler(den_done + 200.0)
            for hh in range(2):
                bcp = ppsum.tile([128, 512], f32, tag="mm512", name="pp")
                nc.tensor.matmul(
                    bcp[0:HD, :],
                    lhsT=ones_sb[0:1, :],
                    rhs=den[:, hh, :],
                )
                bc_done = e_pe(512, dep=den_done, ovh=70.0)
                rcpb = rcpb_pool.tile([HD, 512], f32, tag="rcpb", name="rcpb")
                nc.vector.reciprocal_approx_fast(out=rcpb, in_=bcp[0:HD, :])
                e_dve(512, dep=bc_done)
                nc.vector.scalar_tensor_tensor(
                    out=otn[hh * HD:(hh + 1) * HD, :],
                    in0=pv[0:HD, hh, :],
                    scalar=1.0,
                    in1=rcpb,
                    op0=MUL,
                    op1=MUL,
                )
                e_dve(512)
            # the broadcast matmuls are already queued (gated on den); pad
            # the final reciprocal/scale window so the tail stays warm.
            if ci == nci - 1 and pair == 1:
                emit_warm_dummies(5)
            pv_free[0] = est["dve"] + 100.0
            if pair == 1:
                otn_done[ci] = est["dve"]
                push_outproj(ci)
            elif ci == nci - 1:
                otn_p0_done[ci] = est["dve"]
                for op in range(4):
                    fill_q.append((("op0", ci, op),
                                   lambda ci=ci: otn_p0_done.get(ci, 0.0),
                                   lambda ci=ci, op=op:
                                   emit_outproj_pair(ci, op, 0)))

        # ---- main program ----------------------------------------------
        emit_qk_pass_ci0()
        emitted.add(("qk", 0, 0))
        emitted.add(("qk", 0, 1))
        for it in range(4):
            fill_q.append((("v", 0, it),
                           lambda: max(xin_done[0], wv_done),
                           lambda it=it: emit_v_chunk(0, it)))
        for ft in (2, 3):
            fill_q.append((("qk", 0, ft),
                           lambda: xin_done[0],
                           lambda ft=ft: emit_qk_chunk(0, ft)))
        for ci in range(1, nci):
            push_proj(ci)

        # cover the pass-A -> evac -> first-scores window (both ppsum bufs are
        # held by the pass-A evacuations, so no filler can run here).
        emit_warm_dummies(8)

        for ci in range(nci):
            emit_attn_pair(ci, 0)
            # all proj(ci) chunks are now emitted (forced by pair 0), so the
            # xin buffer that xin(ci+2) reuses has no pending readers left
            # behind in the filler queue.
            if ci + 2 <= nci - 1:
                for it in range(4):
                    pull(("v", ci, it))
                for ft in range(4):
                    pull(("qk", ci, ft))
                issue_xin(ci + 2, nc.sync if ci % 2 == 0 else nc.gpsimd)
            emit_attn_pair(ci, 1)

        # drain whatever filler remains (outproj of the last chunks),
        # padding between chunks so the PE never sits fully idle long
        # enough to re-throttle during the drain.
        while fill_q:
            k, _, emit = fill_q.popleft()
            emitted.add(k)
            emit()
            if fill_q:
                emit_warm_dummies(2)
    nc.compile()
    return nc


def shard_inputs(x, w_qkv, w_out, t=T):
    """Host-side sharding: returns list of 8 in_maps."""
    nci = t // 512
    x = np.asarray(x, dtype=np.float32)
    w_qkv = np.asarray(w_qkv, dtype=np.float32)
    w_out = np.asarray(w_out, dtype=np.float32)
    wq = w_qkv[0:D].reshape(H, HD, D)
    wk = w_qkv[D:2 * D].reshape(H, HD, D)
    wv_ = w_qkv[2 * D:3 * D].reshape(H, HD, D)
    in_maps = []
    for core in range(NCORES):
        b, g = core // 4, core % 4
        hs = [4 * g + i for i in range(HPC)]
        # xt blocks: [ci][p][kt][it] so each xin chunk is one contiguous DMA
        xt = np.ascontiguousarray(
            x[b, :t].T.reshape(8, 128, nci, 512).transpose(2, 1, 0, 3)
            .astype(bf16))
        cols = []
        for pair in range(2):
            hA, hB = hs[2 * pair], hs[2 * pair + 1]
            cols.append(np.concatenate([wq[hA].T, wq[hB].T], axis=1))  # q tile
            cols.append(np.concatenate([wk[hA].T, wk[hB].T], axis=1))  # k tile
        wqk_c = np.concatenate(cols, axis=1)                           # [D, 512]
        wqk_c = wqk_c.reshape(8, 128, 512).transpose(1, 0, 2)          # [p,kt,f]
        wqk_c = np.ascontiguousarray(
            np.stack([wqk_c[:, :, 0:256], wqk_c[:, :, 256:512]]))      # [2,p,kt,256]
        wv_c = np.concatenate([wv_[h].T for h in hs], axis=1)          # [D, 256]
        wv_c = np.ascontiguousarray(
            wv_c.reshape(8, 128, 256).transpose(1, 0, 2))              # [p,kt,f]
        # wo[dd, pair, o] = w_out[o, head(pair, dd//64)*64 + dd%64]
        wo_c = np.ascontiguousarray(np.stack([
            np.concatenate(
                [w_out[:, hs[2 * p] * HD:(hs[2 * p] + 1) * HD].T,
                 w_out[:, hs[2 * p + 1] * HD:(hs[2 * p + 1] + 1) * HD].T],
                axis=0)
            for p in range(2)], axis=1))                               # [128, 2, D]
        in_maps.append({"xt": xt, "wqk": wqk_c.astype(bf16),
                        "wv": wv_c.astype(bf16), "wo": wo_c.astype(bf16)})
    return in_maps


def kernel(x, w_qkv, w_out, _trace=False):
    global LAST_RESULTS
    in_maps = shard_inputs(x, w_qkv, w_out)
    nc = build_bass()
    res = run_bass_kernel_spmd(
        nc, in_maps, core_ids=list(range(NCORES)), trace=_trace
    )
    LAST_RESULTS = res
    out = np.zeros((B, T, D), dtype=np.float32)
    for core in range(NCORES):
        b = core // 4
        # outp blocks: [ci][op][p][(half it)]; o = (2*op+half)*128 + p
        arr = res.results[core]["outp"].reshape(T // 512, 4, 128, 2, 512)
        part = arr.transpose(0, 4, 1, 3, 2).reshape(T, D)
        out[b] += part.astype(np.float32)
        # pair-0 partials of the last i-chunk
        arr2 = res.results[core]["outp2"].reshape(4, 128, 2, 512)
        part2 = arr2.transpose(3, 0, 2, 1).reshape(512, D)
        out[b, T - 512:T] += part2.astype(np.float32)
    return out


# revision 62
# speedup vs baseline: 1.0389x; 1.0389x over previous
"""Causal multi-head self-attention on 8 trn2 NeuronCores (bf16, pipelined).

Sharding: core c = (batch, head_group): batch = c // 4, heads = [4*(c%4) .. 4*(c%4)+3].
Each core computes the QKV projection for its batch + 4 heads, causal attention,
and a row-parallel slice of the output projection; the host sums the 4 partial
outputs per batch element.

Device design notes:
 - all matmul operands are bf16: the PE streams bf16 moving operands at
   1 col/cycle @2.4GHz vs 2 bytes/cycle for fp32r. PSUM accumulation stays fp32.
 - every DRAM tensor is laid out so each dma_start moves one fully
   contiguous block per partition (xt pre-tiled [ci][p][kt][it], weights
   [p][kt][f], outputs [ci][op][p][f]); host does the rearranges.
 - a run of dummy 128-col matmuls on a memset tile warms the PE HAM clock
   gate (1.2 -> 2.4 GHz) during the initial weight/x DMA wait.
 - x is passed transposed so both projection matmuls have the contraction
   dim (channels) on partitions.
 - attention scores are computed transposed: ST[j, i] = (k_j . q_i)/8 with j on
   partitions. The two heads of a pair run as one packed PE slot via
   tile_position (0,0)/(64,0) row tiling (K=64 each).
 - softmax denominator comes from a ones-column appended to V (M=65 PV matmul);
   it is inverted with DVE reciprocal and broadcast across 64 partitions with a
   ones-column PE matmul.
 - no max-subtraction in softmax: scores are ~N(0,1), exp is safe in fp32 PSUM.
 - causal blocks are exact at 128-column granularity; diagonal triangles are
   zeroed by gpsimd affine_select after exp.
 - emission is planned with a coarse per-engine time model: projection and
   output-projection matmul chunks are held in a filler queue and emitted
   wherever the PE queue would otherwise block on a semaphore (exp results,
   PSUM tile reuse), so the PE pipeline stays dense.
"""

import ml_dtypes
import numpy as np
from collections import deque
from contextlib import ExitStack

import concourse.bass as bass
from concourse import bacc
import concourse.mybir as mybir
import concourse.tile as tile
from concourse.bass_utils import run_bass_kernel_spmd

bf16 = ml_dtypes.bfloat16

B, T, D, H, HD = 2, 2048, 1024, 16, 64
NCORES = 8
HPC = 4  # heads per core

f32 = mybir.dt.float32
R = mybir.dt.bfloat16
Exp = mybir.ActivationFunctionType.Exp
MUL = mybir.AluOpType.mult

LAST_RESULTS = None  # BassKernelResults of the most recent kernel() call

N_WARM = 15  # N=512 HAM warmup matmuls: ~6-9 cold (427ns) + rest warm (216ns)


def build_bass(t=T):
    """Build the per-core Bass program (SPMD: same program, different data)."""
    assert t % 512 == 0
    nci = t // 512      # 512-wide i-chunks
    njt_tot = t // 128  # 128-wide j-tiles

    nc = bacc.Bacc("TRN2", target_bir_lowering=False)
    xt = nc.dram_tensor("xt", [nci, 128, 8, 512], R, kind="ExternalInput")
    # wqk split by pair-half (ft 0/1 vs 2/3): pass A only needs half 0, so
    # all of pass A's input rides first-FIFO-slot DMAs on the three queues.
    wqk = nc.dram_tensor("wqk", [2, 128, 8, 256], R, kind="ExternalInput")
    wv = nc.dram_tensor("wv", [128, 8, 256], R, kind="ExternalInput")
    wo = nc.dram_tensor("wo", [128, 2, D], R, kind="ExternalInput")
    outp = nc.dram_tensor("outp", [nci, 4, 128, 1024], R, kind="ExternalOutput")
    # pair-0-only output-projection partials of the last i-chunk: computed
    # during the final pair's attention windows, summed with outp on host.
    outp2 = nc.dram_tensor("outp2", [4, 128, 1024], R, kind="ExternalOutput")

    with ExitStack() as ctx:
        tc = ctx.enter_context(tile.TileContext(nc))
        persist = ctx.enter_context(tc.tile_pool(name="persist", bufs=1))
        xin_pool = ctx.enter_context(tc.tile_pool(name="xin", bufs=2))
        exps = ctx.enter_context(tc.tile_pool(name="exps", bufs=4))
        otn_pool = ctx.enter_context(tc.tile_pool(name="otn", bufs=8))
        rcp_pool = ctx.enter_context(tc.tile_pool(name="rcp", bufs=2))
        rcpb_pool = ctx.enter_context(tc.tile_pool(name="rcpb", bufs=2))
        osb_pool = ctx.enter_context(tc.tile_pool(name="osb", bufs=3))
        ppsum = ctx.enter_context(tc.tile_pool(name="ppsum", bufs=2, space="PSUM"))
        spsum = ctx.enter_context(tc.tile_pool(name="spsum", bufs=2, space="PSUM"))
        pvpsum = ctx.enter_context(tc.tile_pool(name="pvpsum", bufs=1, space="PSUM"))

        # ---- coarse per-engine completion-time estimates (ns) -----------
        est = {"pe": 0.0, "act": 0.0, "dve": 0.0}

        def e_pe(n_cols, dep=0.0, ovh=15.0):
            est["pe"] = max(est["pe"], dep) + n_cols / 2.4 + ovh
            return est["pe"]

        def e_act(fd, dep=0.0):
            est["act"] = max(est["act"], dep + 120.0) + 290.0 + fd / 1.2
            return est["act"]

        def e_dve(fd, dep=0.0, ovh=165.0):
            est["dve"] = max(est["dve"], dep + 120.0) + ovh + fd / 0.96
            return est["dve"]

        def e_dma(nbytes_total):
            return max(est["pe"], est["dve"]) + 2000.0 + nbytes_total / 350.0

        # ---- weights / constants ---------------------------------------
        wqk_sb = persist.tile([128, 2, 8, 256], R, tag="wqk_sb", name="wqk_sb")
        wv_sb = persist.tile([128, 8, 256], R, tag="wv_sb", name="wv_sb")
        wo_sb = persist.tile([128, 2, D], R, tag="wo_sb", name="wo_sb")
        ones_sb = persist.tile([128, 64], R, tag="ones_sb", name="ones_sb")
        warm_sb = persist.tile([128, 512], R, tag="warm_sb", name="warm_sb")

        v_sb = persist.tile([128, njt_tot, HPC, HD + 1], R, tag="v_sb", name="v_sb")

        # qk_sb[ft][ci]: ft 0=q pair0, 1=k pair0, 2=q pair1, 3=k pair1
        # each tile [128, 512]: partitions 0:64 head A dims, 64:128 head B dims
        qk_sb = [[persist.tile([128, 512], R, tag=f"qk_{ft}_{ci}", name=f"qk_{ft}_{ci}")
                  for ci in range(nci)] for ft in range(4)]

        xin_tiles = {}
        xin_done = {}

        def issue_xin(ci, eng):
            if ci in xin_tiles or ci >= nci:
                return
            xin = xin_pool.tile([128, 8, 512], R, tag="xin", name="xin")
            xin_tiles[ci] = xin
            eng.dma_start(out=xin, in_=xt[ci])
            xin_done[ci] = e_dma(128 * 8 * 512 * 2)

        # HAM warmup: memset a tile, then dummy matmuls keep the PE busy
        # while the first weight/x DMAs are in flight.  warm_sb is memset on
        # gpsimd, whose preamble finishes earliest, so the first warm matmul
        # is not gated on the vector queue's slower startup.
        nc.gpsimd.memset(warm_sb, 0.0)
        nc.vector.memset(ones_sb, 1.0)

        # first-needed inputs first, interleaved across the three DMA-capable
        # queues in kt order (proj(0) consumes wqk+xin0 kt-laddered, kt-major).
        # Everything not needed in the first ~15us (wv, wo, xin1) is issued
        # later from a busy queue so it cannot steal SDMA bandwidth from the
        # critical proj(0) feed.
        # half-tensor slabs: 4KB per partition keeps the SDMA descriptors
        # efficient (2KB-per-partition slabs measured only ~160GB/s
        # aggregate); the kt0-3 halves land first and feed pass A's ladder.
        # Per-ring DMAs serialize through their ~2us completion receipt, so
        # the three transfers pass A needs (wqk half 0, both xin0 halves)
        # each ride the first FIFO slot of a queue; wqk half 1 and wv are
        # needed several us later and queue second.
        # all critical loads stay on the two HWDGE queues (sync/scalar):
        # gpsimd(SWDGE)-issued DMAs fire their completion ~4-5us after the
        # transfer instead of ~2us.
        xin0 = xin_pool.tile([128, 8, 512], R, tag="xin", name="xin")
        xin_tiles[0] = xin0
        nc.sync.dma_start(out=wqk_sb[:, 0], in_=wqk[0])
        nc.scalar.dma_start(out=xin0[:, 0:4, :], in_=xt[0, :, 0:4, :])
        nc.sync.dma_start(out=xin0[:, 4:8, :], in_=xt[0, :, 4:8, :])
        nc.sync.dma_start(out=wqk_sb[:, 1], in_=wqk[1])
        xin_done[0] = e_dma(1024 * 1024 + 512 * 1024)
        # wv is first needed by the ci=0 v-chunks around ~18us, xin1 by the
        # proj(1) filler during the pair-(0,1) attention windows (~28us).
        nc.scalar.dma_start(out=wv_sb, in_=wv[:])
        wv_done = e_dma(128 * 8 * 256 * 2)
        issue_xin(1, nc.scalar)
        # third-in-FIFO behind xin-h0 and wv with serialized ~2us receipts.
        xin_done[1] = 20000.0

        wps = ppsum.tile([128, 512], f32, tag="mm512", name="pp")
        for _ in range(N_WARM):
            nc.tensor.matmul(wps, lhsT=warm_sb[:, 0:128], rhs=warm_sb,
                             start=True, stop=True)

        def pad_ldw(target, cap=5):
            # standalone background-buffer weight loads: free PE-side
            # activity (no PSUM write, no hazards) to keep the HAM clock
            # gate warm through short exp-paced windows late in the kernel.
            n = 0
            while est["pe"] < target - 300.0 and n < cap:
                nc.tensor.ldweights(weights=warm_sb[:, 0:128])
                est["pe"] += 110.0
                n += 1

        def emit_warm_dummies(n):
            # keep the PE HAM clock gate warm across a known stall window:
            # dummy matmuls into a scratch spsum tile (its previous readers
            # are long done, so only same-engine WAW ordering applies).
            dmy = spsum.tile([128, 2, 512], f32, tag="sp", name="sp")
            for _ in range(n):
                e_pe(512, ovh=10.0)
                nc.tensor.matmul(dmy[:, 0, :], lhsT=warm_sb[:, 0:128],
                                 rhs=warm_sb, start=True, stop=True)

        # v with appended ones column: [j_in_tile, jt, head, 65]
        nc.vector.tensor_copy(
            out=v_sb[:, :, :, HD],
            in_=ones_sb[:, 0].to_broadcast([128, njt_tot, HPC]),
        )

        qk_done = {}
        v_done = {}
        otn_tiles = {}
        otn_done = {}

        def emit_qk_pass_ci0():
            # kt-major over the first two qk chunks of ci=0: each arriving
            # (wqk, xin0) kt-slab immediately feeds 2x512 matmul cols, so the
            # PE consumption rate matches the DMA arrival rate during the
            # initial ladder crawl instead of stalling chunk-by-chunk.
            xin = xin_tiles[0]
            ps = [ppsum.tile([128, 512], f32, tag="mm512", name="pp")
                  for _ in range(2)]
            for kt in range(8):
                slab = 13000.0 + (kt // 4) * 1400.0
                for ft in range(2):
                    e_pe(512, dep=(slab if ft == 0 else 0.0))
                    nc.tensor.matmul(
                        ps[ft],
                        lhsT=wqk_sb[:, 0, kt, ft * 128:(ft + 1) * 128],
                        rhs=xin[:, kt, :],
                        start=(kt == 0), stop=(kt == 7),
                    )
            for ft in range(2):
                nc.vector.tensor_copy(out=qk_sb[ft][0], in_=ps[ft])
                qk_done[(ft, 0)] = e_dve(512, dep=est["pe"])

        def evac(dst, ps):
            # route PSUM evacuations to whichever of DVE/ACT is further
            # behind; ACT only when clearly idle (protects the exp chain).
            if est["act"] + 1500.0 < est["dve"]:
                nc.scalar.activation(
                    out=dst, in_=ps,
                    func=mybir.ActivationFunctionType.Copy,
                )
                return e_act(512, dep=est["pe"])
            nc.vector.tensor_copy(out=dst, in_=ps)
            return e_dve(512, dep=est["pe"])

        def emit_qk_chunk(ci, ft):
            xin = xin_tiles[ci]
            ps = ppsum.tile([128, 512], f32, tag="mm512", name="pp")
            for kt in range(8):
                e_pe(512, dep=(xin_done.get(ci, 0.0) if kt == 0 else 0.0))
                nc.tensor.matmul(
                    ps,
                    lhsT=wqk_sb[:, ft // 2, kt, (ft % 2) * 128:(ft % 2 + 1) * 128],
                    rhs=xin[:, kt, :],
                    start=(kt == 0), stop=(kt == 7),
                )
            qk_done[(ft, ci)] = evac(qk_sb[ft][ci], ps)

        def emit_v_chunk(ci, it):
            xin = xin_tiles[ci]
            jt = ci * 4 + it
            ps = ppsum.tile([128, 512], f32, tag="mm512", name="pp")
            for kt in range(8):
                e_pe(256, dep=(max(xin_done.get(ci, 0.0), wv_done)
                               if kt == 0 else 0.0))
                nc.tensor.matmul(
                    ps[:, 0:256],
                    lhsT=xin[:, kt, it * 128:(it + 1) * 128],
                    rhs=wv_sb[:, kt, :],
                    start=(kt == 0), stop=(kt == 7),
                )
            nc.vector.tensor_copy(
                out=v_sb[:, jt, :, 0:HD],
                in_=ps[:, 0:256].rearrange("p (h d) -> p h d", h=HPC),
            )
            v_done[jt] = e_dve(256, dep=est["pe"])

        def emit_outproj_chunk(ci, op):
            # one chunk = two adjacent ot halves -> one 256KB contiguous DMA
            osb = osb_pool.tile([128, 1024], R, tag="osb", name="osb")
            for half in range(2):
                ot = 2 * op + half
                ps = ppsum.tile([128, 512], f32, tag="mm512", name="pp")
                for pair in range(2):
                    e_pe(512, dep=(otn_done.get(ci, 0.0)
                                   if (pair == 0 and half == 0) else 0.0))
                    nc.tensor.matmul(
                        ps,
                        lhsT=wo_sb[:, pair, ot * 128:(ot + 1) * 128],
                        rhs=otn_tiles[(ci, pair)],
                        start=(pair == 0), stop=(pair == 1),
                    )
                dst = osb[:, half * 512:(half + 1) * 512]
                evac(dst, ps)
            dma_eng = nc.sync if op % 2 == 0 else nc.gpsimd
            dma_eng.dma_start(out=outp[ci, op], in_=osb)

        otn_p0_done = {}

        def emit_outproj_pair(ci, op, pair):
            # last-chunk output projection split by pair: pair-0 partials run
            # during the final pair's attention windows, pair-1 partials are
            # all that remains after the last normalize; host sums the two.
            osb = osb_pool.tile([128, 1024], R, tag="osb", name="osb")
            for half in range(2):
                ot = 2 * op + half
                ps = ppsum.tile([128, 512], f32, tag="mm512", name="pp")
                e_pe(512, dep=((otn_p0_done.get(ci, 0.0) if pair == 0
                                else otn_done.get(ci, 0.0))
                               if half == 0 else 0.0))
                nc.tensor.matmul(
                    ps,
                    lhsT=wo_sb[:, pair, ot * 128:(ot + 1) * 128],
                    rhs=otn_tiles[(ci, pair)],
                    start=True, stop=True,
                )
                dst = osb[:, half * 512:(half + 1) * 512]
                if pair == 1 and half == 1:
                    # tail: scalar is idle once the last exp is done -
                    # alternating evacuations halves the serialized drain.
                    nc.scalar.activation(
                        out=dst, in_=ps,
                        func=mybir.ActivationFunctionType.Copy,
                    )
                    est["act"] += 600.0
                else:
                    nc.vector.tensor_copy(out=dst, in_=ps)
                    e_dve(512, dep=est["pe"])
            if pair == 0:
                dma_eng = nc.sync if op % 2 == 0 else nc.gpsimd
                dma_eng.dma_start(out=outp2[op], in_=osb)
            else:
                dma_eng = [nc.sync, nc.gpsimd, nc.scalar][op % 3]
                dma_eng.dma_start(out=outp[ci, op], in_=osb)

        # ---- filler queue of PE chunks ---------------------------------
        # each entry: [key, ready_fn, emit_fn]; emitted at most once.
        fill_q = deque()
        emitted = set()

        INF = float("inf")

        def push_proj(ci):
            for ft in range(4):
                fill_q.append((("qk", ci, ft),
                               lambda ci=ci: xin_done.get(ci, INF),
                               lambda ci=ci, ft=ft: emit_qk_chunk(ci, ft)))
            for it in range(4):
                fill_q.append((("v", ci, it),
                               lambda ci=ci: xin_done.get(ci, INF),
                               lambda ci=ci, it=it: emit_v_chunk(ci, it)))

        def push_outproj(ci):
            if ci == nci - 1:
                for op in range(4):
                    fill_q.append((("op1", ci, op),
                                   lambda ci=ci: otn_done.get(ci, 0.0),
                                   lambda ci=ci, op=op:
                                   emit_outproj_pair(ci, op, 1)))
            else:
                for op in range(4):
                    fill_q.append((("op", ci, op),
                                   lambda ci=ci: otn_done.get(ci, 0.0),
                                   lambda ci=ci, op=op:
                                   emit_outproj_chunk(ci, op)))

        def pull(key):
            """Force-emit a specific chunk now (if not already emitted)."""
            if key in emitted:
                return
            for i, (k, _, emit) in enumerate(fill_q):
                if k == key:
                    del fill_q[i]
                    emitted.add(k)
                    emit()
                    return

        def pull_filler(target, keep=0):
            """Emit ready filler chunks until est pe time reaches target."""
            while len(fill_q) > keep and est["pe"] < target:
                picked = None
                for i, (k, ready, _) in enumerate(fill_q):
                    if ready() <= est["pe"] + 100.0:
                        picked = i
                        break
                if picked is None:
                    break
                k, _, emit = fill_q[picked]
                del fill_q[picked]
                emitted.add(k)
                emit()

        # ---- attention -------------------------------------------------
        def emit_scores(ci, pair, jt):
            pull(("qk", ci, 2 * pair))
            pull(("qk", jt // 4, 2 * pair + 1))
            s = max(0, (jt - 4 * ci)) * 128
            qtile = qk_sb[2 * pair][ci]
            ktile = qk_sb[2 * pair + 1][jt // 4]
            ksl = ktile[:, (jt % 4) * 128:(jt % 4 + 1) * 128]
            sp = spsum.tile([128, 2, 512], f32, tag="sp", name="sp")
            dep = max(qk_done.get((2 * pair, ci), 0.0),
                      qk_done.get((2 * pair + 1, jt // 4), 0.0))
            nc.tensor.matmul(
                sp[:, 0, s:512],
                lhsT=ksl[0:64, :],
                rhs=qtile[0:64, s:512],
                tile_position=(0, 0),
            )
            nc.tensor.matmul(
                sp[:, 1, s:512],
                lhsT=ksl[64:128, :],
                rhs=qtile[64:128, s:512],
                tile_position=(64, 0),
            )
            sp_done = e_pe(512 - s, dep=dep, ovh=120.0)
            return sp, s, sp_done

        def emit_exp(ci, pair, jt, sp, s, sp_done):
            ex = exps.tile([128, 2, 512], R, tag="ex", name="ex")
            nc.scalar.activation(
                out=ex[:, :, s:512], in_=sp[:, :, s:512],
                func=Exp, scale=0.125,
            )
            ex_done = e_act(2 * (512 - s), dep=sp_done)
            if jt - 4 * ci >= 0:
                # zero the diagonal triangle, both heads in one strided call
                nc.gpsimd.affine_select(
                    out=ex[:, :, s:s + 128],
                    in_=ex[:, :, s:s + 128],
                    compare_op=mybir.AluOpType.is_ge,
                    fill=0.0,
                    base=0,
                    channel_multiplier=-1,
                    pattern=[[0, 2], [1, 128]],
                )
                ex_done += 480.0
            return ex, ex_done

        def emit_pv(ci, pair, jt, njt, pv, ex, s, ex_done):
            for hh in range(2):
                e_pe(512 - s, dep=(max(ex_done, v_done.get(jt, 0.0))
                                   if hh == 0 else 0.0), ovh=8.0)
                nc.tensor.matmul(
                    pv[:, hh, s:512],
                    lhsT=v_sb[:, jt, 2 * pair + hh, :],
                    rhs=ex[:, hh, s:512],
                    start=(jt == 0), stop=(jt == njt - 1),
                )

        pv_free = [0.0]

        def emit_attn_pair(ci, pair):
            njt = 4 * (ci + 1)
            pv = pvpsum.tile([HD + 1, 2, 512], f32, tag="pv", name="pv")
            sps = {0: emit_scores(ci, pair, 0)}
            exs = {}
            # hold a couple of filler chunks back during the final pair so the
            # PE is not left fully idle (HAM re-throttle to 1.2GHz) in the
            # last normalize window before the tail output projection.
            keep = 2 if (ci == nci - 1 and pair == 1) else 0
            for jt in range(njt):
                sp, s, sp_done = sps.pop(jt)
                exs[jt] = (emit_exp(ci, pair, jt, sp, s, sp_done), s)
                if ci == 0 and pair == 0 and jt == 2:
                    nc.gpsimd.dma_start(out=wo_sb, in_=wo[:])
                if jt + 1 < njt:
                    sps[jt + 1] = emit_scores(ci, pair, jt + 1)
                # PV runs one jt behind its exp so the PE queue head never
                # blocks on the activation; filler fills the remaining slack.
                for j in ([jt - 1, jt] if jt == njt - 1 else [jt - 1]):
                    if j < 0:
                        continue
                    (ex, ex_done), s_j = exs.pop(j)
                    target = ex_done
                    if j == 0:
                        target = max(target, pv_free[0] + 1500.0)
                    pull_filler(target, keep=keep)
                    pull(("v", j // 4, j % 4))
                    emit_pv(ci, pair, j, njt, pv, ex, s_j, ex_done)

            if ci == nci - 1 and pair == 1:
                # final normalize window: the reserved pair-0 outproj chunks
                # plus a few dummies keep the HAM gate warm into the tail.
                emit_warm_dummies(3)
            # normalize: den row -> bf16 -> PE ones-broadcast -> reciprocal
            # -> scale.  (DMA cannot stride-0 broadcast across partitions.)
            # The den row lives on a single partition, so the copy runs
            # serially in one lane; splitting the two heads across DVE and
            # ACT (idle here, between pairs) halves the chain latency.
            den = rcp_pool.tile([1, 2, 512], R, tag="den", name="den")
            nc.vector.tensor_copy(out=den[:, 0, :], in_=pv[HD:HD + 1, 0, :])
            den_dones = [e_dve(512, dep=est["pe"])]
            nc.scalar.activation(
                out=den[:, 1, :], in_=pv[HD:HD + 1, 1, :],
                func=mybir.ActivationFunctionType.Copy,
            )
            den_dones.append(e_act(512, dep=est["pe"]))
            den_done = max(den_dones)
            otn = otn_pool.tile([128, 512], R, tag="otn", name="otn")
            otn_tiles[(ci, pair)] = otn
            # keep the PE fed while the den cast completes: the first
            # broadcast matmul below would otherwise block the queue head.
            pull_filler(den_done + 200.0)
            for hh in range(2):
                bcp = ppsum.tile([128, 512], f32, tag="mm512", name="pp")
                nc.tensor.matmul(
                    bcp[0:HD, :],
                    lhsT=ones_sb[0:1, :],
                    rhs=den[:, hh, :],
                )
                bc_done = e_pe(512, dep=den_dones[hh], ovh=70.0)
                rcpb = rcpb_pool.tile([HD, 512], f32, tag="rcpb", name="rcpb")
                nc.vector.reciprocal_approx_fast(out=rcpb, in_=bcp[0:HD, :])
                e_dve(512, dep=bc_done)
                nc.vector.scalar_tensor_tensor(
                    out=otn[hh * HD:(hh + 1) * HD, :],
                    in0=pv[0:HD, hh, :],
                    scalar=1.0,
                    in1=rcpb,
                    op0=MUL,
                    op1=MUL,
                )
                e_dve(512)
            # the broadcast matmuls are already queued (gated on den); pad
            # the final reciprocal/scale window so the tail stays warm.
            if ci == nci - 1 and pair == 1:
                emit_warm_dummies(5)
            pv_free[0] = est["dve"] + 100.0
            if pair == 1:
                otn_done[ci] = est["dve"]
                push_outproj(ci)
            elif ci == nci - 1:
                otn_p0_done[ci] = est["dve"]
                for op in range(4):
                    fill_q.append((("op0", ci, op),
                                   lambda ci=ci: otn_p0_done.get(ci, 0.0),
                                   lambda ci=ci, op=op:
                                   emit_outproj_pair(ci, op, 0)))

        # ---- main program ----------------------------------------------
        emit_qk_pass_ci0()
        emitted.add(("qk", 0, 0))
        emitted.add(("qk", 0, 1))
        for it in range(4):
            fill_q.append((("v", 0, it),
                           lambda: max(xin_done[0], wv_done),
                           lambda it=it: emit_v_chunk(0, it)))
        for ft in (2, 3):
            fill_q.append((("qk", 0, ft),
                           lambda: xin_done[0],
                           lambda ft=ft: emit_qk_chunk(0, ft)))
        for ci in range(1, nci):
            push_proj(ci)

        # cover the pass-A -> evac -> first-scores window (both ppsum bufs are
        # held by the pass-A evacuations, so no filler can run here).
        emit_warm_dummies(8)

        for ci in range(nci):
            emit_attn_pair(ci, 0)
            # all proj(ci) chunks are now emitted (forced by pair 0), so the
            # xin buffer that xin(ci+2) reuses has no pending readers left
            # behind in the filler queue.
            if ci + 2 <= nci - 1:
                for it in range(4):
                    pull(("v", ci, it))
                for ft in range(4):
                    pull(("qk", ci, ft))
                issue_xin(ci + 2, nc.sync if ci % 2 == 0 else nc.gpsimd)
            emit_attn_pair(ci, 1)

        # drain whatever filler remains (outproj of the last chunks),
        # padding between chunks so the PE never sits fully idle long
        # enough to re-throttle during the drain.
        while fill_q:
            k, _, emit = fill_q.popleft()
            emitted.add(k)
            emit()
            if fill_q:
                emit_warm_dummies(2)
    nc.compile()
    return nc


def shard_inputs(x, w_qkv, w_out, t=T):
    """Host-side sharding: returns list of 8 in_maps."""
    nci = t // 512
    x = np.asarray(x, dtype=np.float32)
    w_qkv = np.asarray(w_qkv, dtype=np.float32)
    w_out = np.asarray(w_out, dtype=np.float32)
    wq = w_qkv[0:D].reshape(H, HD, D)
    wk = w_qkv[D:2 * D].reshape(H, HD, D)
    wv_ = w_qkv[2 * D:3 * D].reshape(H, HD, D)
    in_maps = []
    for core in range(NCORES):
        b, g = core // 4, core % 4
        hs = [4 * g + i for i in range(HPC)]
        # xt blocks: [ci][p][kt][it] so each xin chunk is one contiguous DMA
        xt = np.ascontiguousarray(
            x[b, :t].T.reshape(8, 128, nci, 512).transpose(2, 1, 0, 3)
            .astype(bf16))
        cols = []
        for pair in range(2):
            hA, hB = hs[2 * pair], hs[2 * pair + 1]
            cols.append(np.concatenate([wq[hA].T, wq[hB].T], axis=1))  # q tile
            cols.append(np.concatenate([wk[hA].T, wk[hB].T], axis=1))  # k tile
        wqk_c = np.concatenate(cols, axis=1)                           # [D, 512]
        wqk_c = wqk_c.reshape(8, 128, 512).transpose(1, 0, 2)          # [p,kt,f]
        wqk_c = np.ascontiguousarray(
            np.stack([wqk_c[:, :, 0:256], wqk_c[:, :, 256:512]]))      # [2,p,kt,256]
        wv_c = np.concatenate([wv_[h].T for h in hs], axis=1)          # [D, 256]
        wv_c = np.ascontiguousarray(
            wv_c.reshape(8, 128, 256).transpose(1, 0, 2))              # [p,kt,f]
        # wo[dd, pair, o] = w_out[o, head(pair, dd//64)*64 + dd%64]
        wo_c = np.ascontiguousarray(np.stack([
            np.concatenate(
                [w_out[:, hs[2 * p] * HD:(hs[2 * p] + 1) * HD].T,
                 w_out[:, hs[2 * p + 1] * HD:(hs[2 * p + 1] + 1) * HD].T],
                axis=0)
            for p in range(2)], axis=1))                               # [128, 2, D]
        in_maps.append({"xt": xt, "wqk": wqk_c.astype(bf16),
                        "wv": wv_c.astype(bf16), "wo": wo_c.astype(bf16)})
    return in_maps


def kernel(x, w_qkv, w_out, _trace=False):
    global LAST_RESULTS
    in_maps = shard_inputs(x, w_qkv, w_out)
    nc = build_bass()
    res = run_bass_kernel_spmd(
        nc, in_maps, core_ids=list(range(NCORES)), trace=_trace
    )
    LAST_RESULTS = res
    out = np.zeros((B, T, D), dtype=np.float32)
    for core in range(NCORES):
        b = core // 4
        # outp blocks: [ci][op][p][(half it)]; o = (2*op+half)*128 + p
        arr = res.results[core]["outp"].reshape(T // 512, 4, 128, 2, 512)
        part = arr.transpose(0, 4, 1, 3, 2).reshape(T, D)
        out[b] += part.astype(np.float32)
        # pair-0 partials of the last i-chunk
        arr2 = res.results[core]["outp2"].reshape(4, 128, 2, 512)
        part2 = arr2.transpose(3, 0, 2, 1).reshape(512, D)
        out[b, T - 512:T] += part2.astype(np.float32)
    return out
